# revision 1
# baseline (speedup 1.0000x reference)
"""Trainium2 Bass kernel for nn_BertMoEClassifier.

Full-input contract: kernel(**inputs) takes the unsharded numpy inputs and
returns the full [32, 512, 2] logits.  Internally: data-parallel over the
batch dim across 8 NeuronCores (4 batches = 2048 tokens per core), dense
8-expert MoE with combine-weight masking on-device, no collectives.

Shapes (hardcoded): B=32 S=512 C=3072 D=768 H=1024 E=8 K=2 L=2.

Numerics: the projection matmul and the router run in full fp32 (the
discrete top-2 routing amplifies tiny numeric differences into expert
flips); the expert MLPs run in bf16 with fp32 PSUM accumulation.
"""

from contextlib import ExitStack

import ml_dtypes
import numpy as np

import concourse.bacc as bacc
import concourse.bass as bass
import concourse.mybir as mybir
import concourse.tile as tile
from concourse import bass_utils
from concourse.masks import make_identity

F32 = mybir.dt.float32
BF16 = mybir.dt.bfloat16
AF = mybir.ActivationFunctionType
OP = mybir.AluOpType

B, S, C, D, H, E, L = 32, 512, 3072, 768, 1024, 8, 2
NCORES = 8
T = (B // NCORES) * S            # 2048 tokens per core
NT = T // 128                    # 16 token tiles
KC = C // 128                    # 24 contraction chunks (proj)
KD = D // 128                    # 6 chunks of D
KH = H // 128                    # 8 chunks of H
NEG_BIG = -1.0e30
EPS = 1e-5

_CACHE = {}


def _bcast_row(h_ap, off, n):
    """AP broadcasting a DRAM row of n elements across 128 partitions."""
    return bass.AP(tensor=h_ap.tensor, offset=h_ap.offset + off, ap=[[0, 128], [1, n]])


def _build():
    nc = bacc.Bacc("TRN2", target_bir_lowering=False, debug=False)

    hT_d = nc.dram_tensor("hT", [C, T], F32, kind="ExternalInput")
    pw_d = nc.dram_tensor("pw", [C, D], F32, kind="ExternalInput")
    pb_d = nc.dram_tensor("pb", [D], F32, kind="ExternalInput")
    g1_d = nc.dram_tensor("g1", [D], F32, kind="ExternalInput")
    be1_d = nc.dram_tensor("be1", [D], F32, kind="ExternalInput")
    g2_d = nc.dram_tensor("g2", [D], F32, kind="ExternalInput")
    be2_d = nc.dram_tensor("be2", [D], F32, kind="ExternalInput")
    gw_d = nc.dram_tensor("gw", [128, KD, E], F32, kind="ExternalInput")
    gb_d = nc.dram_tensor("gb", [E], F32, kind="ExternalInput")
    w1_d = nc.dram_tensor("w1", [E, KD, 128, H], BF16, kind="ExternalInput")
    b1_d = nc.dram_tensor("b1", [128, E, KH], F32, kind="ExternalInput")
    w2_d = nc.dram_tensor("w2", [E, KH, 128, D], BF16, kind="ExternalInput")
    b2_d = nc.dram_tensor("b2", [E, D], F32, kind="ExternalInput")
    cwT_d = nc.dram_tensor("cwT", [L, D], F32, kind="ExternalInput")
    cb_d = nc.dram_tensor("cb", [L], F32, kind="ExternalInput")
    out_d = nc.dram_tensor("out", [T, L], F32, kind="ExternalOutput")

    with ExitStack() as ctx:
        tc = ctx.enter_context(tile.TileContext(nc))
        persist = ctx.enter_context(tc.tile_pool(name="persist", bufs=1))

        # ---- persistent tiles -------------------------------------------
        acc = [persist.tile([128, D], F32, name=f"acc{t}", tag=f"acc{t}")
               for t in range(NT)]
        seqT = [persist.tile([128, T], BF16, name=f"seqT{j}", tag=f"seqT{j}")
                for j in range(KD)]
        comb = [persist.tile([128, E], F32, name=f"comb{t}", tag=f"comb{t}")
                for t in range(NT)]
        pbb = persist.tile([128, D], F32, name="pbb", tag="pbb")
        g1b = persist.tile([128, D], F32, name="g1b", tag="g1b")
        be1b = persist.tile([128, D], F32, name="be1b", tag="be1b")
        ident = persist.tile([128, 128], F32, name="ident", tag="ident")
        gwsb = persist.tile([128, KD, E], F32, name="gwsb", tag="gwsb")
        gbb = persist.tile([128, E], F32, name="gbb", tag="gbb")
        b1sb = persist.tile([128, E, KH], F32, name="b1sb", tag="b1sb")
        b2sb = persist.tile([E, D], F32, name="b2sb", tag="b2sb")
        epst = persist.tile([128, 1], F32, name="epst", tag="epst")

        nc.sync.dma_start(out=pbb, in_=_bcast_row(pb_d.ap(), 0, D))
        nc.sync.dma_start(out=g1b, in_=_bcast_row(g1_d.ap(), 0, D))
        nc.sync.dma_start(out=be1b, in_=_bcast_row(be1_d.ap(), 0, D))
        nc.sync.dma_start(out=gwsb, in_=gw_d.ap())
        nc.sync.dma_start(out=gbb, in_=_bcast_row(gb_d.ap(), 0, E))
        nc.sync.dma_start(out=b1sb, in_=b1_d.ap())
        nc.sync.dma_start(out=b2sb, in_=b2_d.ap())
        nc.vector.memset(epst, EPS)
        make_identity(nc, ident)

        groups = [(0, 3), (3, 3), (6, 3), (9, 3), (12, 3), (15, 1)]

        # ================= Phase 1: proj + LN1 + GELU + router ===========
        with tc.tile_pool(name="p1pw", bufs=1) as pwpool, \
             tc.tile_pool(name="p1ht", bufs=6) as htpool, \
             tc.tile_pool(name="p1st", bufs=2) as stgpool, \
             tc.tile_pool(name="p1sm", bufs=4) as smpool, \
             tc.tile_pool(name="p1v", bufs=6) as vpool, \
             tc.tile_pool(name="p1psA", bufs=3, space="PSUM") as psA, \
             tc.tile_pool(name="p1psB", bufs=3, space="PSUM") as psB, \
             tc.tile_pool(name="p1psT", bufs=2, space="PSUM") as psT:

            pw = []
            for k in range(KC):
                pwt = pwpool.tile([128, D], F32, name=f"pw{k}", tag=f"pw{k}")
                nc.sync.dma_start(out=pwt, in_=pw_d.ap()[k * 128:(k + 1) * 128, :])
                pw.append(pwt)

            for g0, gn in groups:
                pa = {}
                pb_ = {}
                for t in range(g0, g0 + gn):
                    pa[t] = psA.tile([128, 512], F32, name=f"pa{t}", tag="psA")
                    pb_[t] = psB.tile([128, 256], F32, name=f"pb{t}", tag="psB")
                for k in range(KC):
                    htt = htpool.tile([128, gn * 128], F32, name=f"ht{g0}_{k}",
                                      tag="ht")
                    nc.sync.dma_start(
                        out=htt,
                        in_=hT_d.ap()[k * 128:(k + 1) * 128,
                                      g0 * 128:(g0 + gn) * 128])
                    st = (k == 0)
                    sp = (k == KC - 1)
                    for i, t in enumerate(range(g0, g0 + gn)):
                        lhs = htt[:, i * 128:(i + 1) * 128]
                        nc.tensor.matmul(pa[t], lhs, pw[k][:, 0:512],
                                         start=st, stop=sp)
                        nc.tensor.matmul(pb_[t], lhs, pw[k][:, 512:768],
                                         start=st, stop=sp)

                for t in range(g0, g0 + gn):
                    x = acc[t]
                    # x = psum + proj_b
                    nc.vector.tensor_tensor(out=x[:, 0:512], in0=pa[t],
                                            in1=pbb[:, 0:512], op=OP.add)
                    nc.vector.tensor_tensor(out=x[:, 512:768], in0=pb_[t],
                                            in1=pbb[:, 512:768], op=OP.add)
                    # LN1 stats
                    stats = smpool.tile([128, 3, 6], F32, name=f"st{t}",
                                        tag="stats")
                    for sg in range(3):
                        nc.vector.bn_stats(out=stats[:, sg, :],
                                           in_=x[:, sg * 256:(sg + 1) * 256])
                    mv = smpool.tile([128, 2], F32, name=f"mv{t}", tag="mv")
                    nc.vector.bn_aggr(out=mv, in_=stats)
                    sd = smpool.tile([128, 1], F32, name=f"sd{t}", tag="sd")
                    nc.scalar.activation(out=sd, in_=mv[:, 1:2], func=AF.Sqrt,
                                         bias=epst, scale=1.0)
                    rstd = smpool.tile([128, 1], F32, name=f"rs{t}", tag="rstd")
                    nc.vector.reciprocal(out=rstd, in_=sd)
                    # xhat = (x - mean) * rstd ; then * g + b
                    nc.vector.tensor_scalar(out=x, in0=x, scalar1=mv[:, 0:1],
                                            scalar2=rstd, op0=OP.subtract,
                                            op1=OP.mult)
                    nc.vector.tensor_tensor(out=x, in0=x, in1=g1b, op=OP.mult)
                    nc.vector.tensor_tensor(out=x, in0=x, in1=be1b, op=OP.add)
                    # seq_out = gelu(x)  (exact erf-based table)
                    nc.scalar.activation(out=x, in_=x, func=AF.Gelu)

                    # transpose to fp32 staging + bf16 seqT
                    stg = stgpool.tile([128, KD, 128], F32, name=f"stg{t}",
                                       tag="stg")
                    for j in range(KD):
                        pt = psT.tile([128, 128], F32, name=f"pt{t}_{j}",
                                      tag="psT")
                        nc.tensor.transpose(pt, x[:, j * 128:(j + 1) * 128],
                                            ident)
                        nc.scalar.copy(out=stg[:, j, :], in_=pt)
                        nc.vector.tensor_copy(
                            out=seqT[j][:, t * 128:(t + 1) * 128],
                            in_=stg[:, j, :])

                    # router logits (full fp32)
                    pr = psT.tile([128, E], F32, name=f"pr{t}", tag="psT")
                    for j in range(KD):
                        nc.tensor.matmul(pr, stg[:, j, :], gwsb[:, j, :],
                                         start=(j == 0), stop=(j == KD - 1))
                    lg = vpool.tile([128, E], F32, name=f"lg{t}", tag="lg")
                    nc.vector.tensor_tensor(out=lg, in0=pr, in1=gbb, op=OP.add)
                    # top-2 renormalized combine weights
                    m1 = smpool.tile([128, 1], F32, name=f"m1{t}", tag="m1")
                    nc.vector.reduce_max(out=m1, in_=lg, axis=mybir.AxisListType.X)
                    nm1 = smpool.tile([128, 1], F32, name=f"nm1{t}", tag="nm1")
                    nc.vector.tensor_scalar_mul(out=nm1, in0=m1, scalar1=-1.0)
                    ea = vpool.tile([128, E], F32, name=f"ea{t}", tag="ea")
                    nc.scalar.activation(out=ea, in_=lg, func=AF.Exp, bias=nm1,
                                         scale=1.0)
                    mm = vpool.tile([128, E], F32, name=f"mm{t}", tag="mm")
                    nc.vector.tensor_scalar(out=mm, in0=lg, scalar1=m1,
                                            scalar2=None, op0=OP.is_ge)
                    lg2 = vpool.tile([128, E], F32, name=f"lg2{t}", tag="lg2")
                    nc.vector.scalar_tensor_tensor(out=lg2, in0=mm,
                                                   scalar=NEG_BIG, in1=lg,
                                                   op0=OP.mult, op1=OP.add)
                    m2 = smpool.tile([128, 1], F32, name=f"m2{t}", tag="m2")
                    nc.vector.reduce_max(out=m2, in_=lg2, axis=mybir.AxisListType.X)
                    mk2 = vpool.tile([128, E], F32, name=f"mk2{t}", tag="mk2")
                    nc.vector.tensor_scalar(out=mk2, in0=lg, scalar1=m2,
                                            scalar2=None, op0=OP.is_ge)
                    p2 = vpool.tile([128, E], F32, name=f"p2{t}", tag="p2")
                    nc.vector.tensor_mul(out=p2, in0=ea, in1=mk2)
                    sm = smpool.tile([128, 1], F32, name=f"sm{t}", tag="sm")
                    nc.vector.reduce_sum(out=sm, in_=p2, axis=mybir.AxisListType.X)
                    rsm = smpool.tile([128, 1], F32, name=f"rsm{t}", tag="rsm")
                    nc.vector.reciprocal(out=rsm, in_=sm)
                    nc.vector.tensor_scalar_mul(out=comb[t], in0=p2, scalar1=rsm)

        # ================= Phase 2: dense 8-expert MoE ===================
        with tc.tile_pool(name="p2w1", bufs=8) as w1pool, \
             tc.tile_pool(name="p2w2", bufs=10) as w2pool, \
             tc.tile_pool(name="p2h", bufs=18) as hpool, \
             tc.tile_pool(name="p2psA", bufs=3, space="PSUM") as psA2, \
             tc.tile_pool(name="p2psE", bufs=2, space="PSUM") as psE, \
             tc.tile_pool(name="p2psB", bufs=2, space="PSUM") as psB2:

            for e in range(E):
                w1t = []
                for k in range(KD):
                    w = w1pool.tile([128, H], BF16, name=f"w1_{e}_{k}", tag="w1")
                    nc.sync.dma_start(out=w, in_=w1_d.ap()[e, k])
                    w1t.append(w)
                w2t = []
                for k in range(KH):
                    w = w2pool.tile([128, D], BF16, name=f"w2_{e}_{k}", tag="w2")
                    nc.sync.dma_start(out=w, in_=w2_d.ap()[e, k])
                    w2t.append(w)

                for n in range(T // 512):
                    htiles = []
                    for m in range(KH):
                        ps = psA2.tile([128, 512], F32, name=f"ph{e}_{n}_{m}",
                                       tag="psA2")
                        for k in range(KD):
                            nc.tensor.matmul(
                                ps, w1t[k][:, m * 128:(m + 1) * 128],
                                seqT[k][:, n * 512:(n + 1) * 512],
                                start=(k == 0), stop=(k == KD - 1))
                        h = hpool.tile([128, 512], BF16, name=f"h{e}_{n}_{m}",
                                       tag="h")
                        nc.scalar.activation(out=h, in_=ps, func=AF.Gelu,
                                             bias=b1sb[:, e:e + 1, m:m + 1],
                                             scale=1.0)
                        htiles.append(h)
                    for ti in range(4):
                        t = n * 4 + ti
                        pea = psE.tile([128, 512], F32, name=f"pea{e}_{t}",
                                       tag="psE")
                        peb = psB2.tile([128, 256], F32, name=f"peb{e}_{t}",
                                        tag="psB2")
                        for k in range(KH):
                            lhs = htiles[k][:, ti * 128:(ti + 1) * 128]
                            nc.tensor.matmul(pea, lhs, w2t[k][:, 0:512],
                                             start=(k == 0), stop=(k == KH - 1))
                            nc.tensor.matmul(peb, lhs, w2t[k][:, 512:768],
                                             start=(k == 0), stop=(k == KH - 1))
                        c = comb[t][:, e:e + 1]
                        nc.vector.scalar_tensor_tensor(
                            out=acc[t][:, 0:512], in0=pea, scalar=c,
                            in1=acc[t][:, 0:512], op0=OP.mult, op1=OP.add)
                        nc.vector.scalar_tensor_tensor(
                            out=acc[t][:, 512:768], in0=peb, scalar=c,
                            in1=acc[t][:, 512:768], op0=OP.mult, op1=OP.add)

        # ================= Phase 3: +combine@b2, LN2, classifier =========
        with tc.tile_pool(name="p3", bufs=2) as p3pool, \
             tc.tile_pool(name="p3sm", bufs=4) as sm3, \
             tc.tile_pool(name="p3out", bufs=4) as outpool, \
             tc.tile_pool(name="p3psT", bufs=2, space="PSUM") as psT3, \
             tc.tile_pool(name="p3psA", bufs=2, space="PSUM") as psA3, \
             tc.tile_pool(name="p3psB", bufs=2, space="PSUM") as psB3:

            g2b = p3pool.tile([128, D], F32, name="g2b", tag="g2b", bufs=1)
            be2b = p3pool.tile([128, D], F32, name="be2b", tag="be2b", bufs=1)
            cw0 = p3pool.tile([128, D], F32, name="cw0", tag="cw0", bufs=1)
            cw1 = p3pool.tile([128, D], F32, name="cw1", tag="cw1", bufs=1)
            cbb = p3pool.tile([128, L], F32, name="cbb", tag="cbb", bufs=1)
            nc.sync.dma_start(out=g2b, in_=_bcast_row(g2_d.ap(), 0, D))
            nc.sync.dma_start(out=be2b, in_=_bcast_row(be2_d.ap(), 0, D))
            nc.sync.dma_start(out=cw0, in_=_bcast_row(cwT_d.ap(), 0, D))
            nc.sync.dma_start(out=cw1, in_=_bcast_row(cwT_d.ap(), D, D))
            nc.sync.dma_start(out=cbb, in_=_bcast_row(cb_d.ap(), 0, L))

            for t in range(NT):
                x = acc[t]
                # acc += combine[t] @ b2   (fold per-expert output bias)
                ptc = psT3.tile([E, 128], F32, name=f"ptc{t}", tag="psT3")
                nc.tensor.transpose(ptc, comb[t], ident)
                cT = sm3.tile([E, 128], F32, name=f"cT{t}", tag="cT")
                nc.scalar.copy(out=cT, in_=ptc)
                pca = psA3.tile([128, 512], F32, name=f"pca{t}", tag="psA3")
                pcb = psB3.tile([128, 256], F32, name=f"pcb{t}", tag="psB3")
                nc.tensor.matmul(pca, cT, b2sb[:, 0:512], start=True, stop=True)
                nc.tensor.matmul(pcb, cT, b2sb[:, 512:768], start=True, stop=True)
                nc.vector.scalar_tensor_tensor(out=x[:, 0:512], in0=pca,
                                               scalar=1.0, in1=x[:, 0:512],
                                               op0=OP.mult, op1=OP.add)
                nc.vector.scalar_tensor_tensor(out=x[:, 512:768], in0=pcb,
                                               scalar=1.0, in1=x[:, 512:768],
                                               op0=OP.mult, op1=OP.add)
                # LN2
                stats = sm3.tile([128, 3, 6], F32, name=f"s3{t}", tag="s3")
                for sg in range(3):
                    nc.vector.bn_stats(out=stats[:, sg, :],
                                       in_=x[:, sg * 256:(sg + 1) * 256])
                mv = sm3.tile([128, 2], F32, name=f"mv3{t}", tag="mv3")
                nc.vector.bn_aggr(out=mv, in_=stats)
                sd = sm3.tile([128, 1], F32, name=f"sd3{t}", tag="sd3")
                nc.scalar.activation(out=sd, in_=mv[:, 1:2], func=AF.Sqrt,
                                     bias=epst, scale=1.0)
                rstd = sm3.tile([128, 1], F32, name=f"rs3{t}", tag="rs3")
                nc.vector.reciprocal(out=rstd, in_=sd)
                nc.vector.tensor_scalar(out=x, in0=x, scalar1=mv[:, 0:1],
                                        scalar2=rstd, op0=OP.subtract,
                                        op1=OP.mult)
                nc.vector.tensor_tensor(out=x, in0=x, in1=g2b, op=OP.mult)
                nc.vector.tensor_tensor(out=x, in0=x, in1=be2b, op=OP.add)
                # logits = y @ cls_w + cls_b  (DVE dot products)
                lt = outpool.tile([128, L], F32, name=f"lt{t}", tag="lt")
                scr = p3pool.tile([128, D], F32, name=f"scr{t}", tag="scr")
                nc.vector.scalar_tensor_tensor(out=scr, in0=x, scalar=1.0,
                                               in1=cw0, op0=OP.mult,
                                               op1=OP.mult,
                                               accum_out=lt[:, 0:1])
                scr2 = p3pool.tile([128, D], F32, name=f"scr2{t}", tag="scr2")
                nc.vector.scalar_tensor_tensor(out=scr2, in0=x, scalar=1.0,
                                               in1=cw1, op0=OP.mult,
                                               op1=OP.mult,
                                               accum_out=lt[:, 1:2])
                nc.vector.tensor_tensor(out=lt, in0=lt, in1=cbb, op=OP.add)
                nc.sync.dma_start(out=out_d.ap()[t * 128:(t + 1) * 128, :],
                                  in_=lt)

    nc.compile()
    nc.finalize()
    return nc


def _get_nc():
    if "nc" not in _CACHE:
        _CACHE["nc"] = _build()
    return _CACHE["nc"]


def _prep_maps(hidden_states, proj_w, proj_b, ln1_g, ln1_b, gate_w, gate_b,
               w1, b1, w2, b2, ln2_g, ln2_b, cls_w, cls_b):
    bf16 = ml_dtypes.bfloat16
    f32 = np.float32
    shared = {
        "pw": np.ascontiguousarray(proj_w, dtype=f32),
        "pb": np.ascontiguousarray(proj_b, dtype=f32),
        "g1": np.ascontiguousarray(ln1_g, dtype=f32),
        "be1": np.ascontiguousarray(ln1_b, dtype=f32),
        "g2": np.ascontiguousarray(ln2_g, dtype=f32),
        "be2": np.ascontiguousarray(ln2_b, dtype=f32),
        # gate_w [D,E] -> [128, KD, E]
        "gw": np.ascontiguousarray(
            np.asarray(gate_w, dtype=f32).reshape(KD, 128, E).transpose(1, 0, 2)),
        "gb": np.ascontiguousarray(gate_b, dtype=f32),
        # w1 [E,D,H] -> [E, KD, 128, H] bf16
        "w1": np.ascontiguousarray(
            np.asarray(w1).reshape(E, KD, 128, H)).astype(bf16),
        # b1 [E,H] -> [128, E, KH]
        "b1": np.ascontiguousarray(
            np.asarray(b1, dtype=f32).reshape(E, KH, 128).transpose(2, 0, 1)),
        # w2 [E,H,D] -> [E, KH, 128, D] bf16
        "w2": np.ascontiguousarray(
            np.asarray(w2).reshape(E, KH, 128, D)).astype(bf16),
        "b2": np.ascontiguousarray(b2, dtype=f32),
        "cwT": np.ascontiguousarray(np.asarray(cls_w, dtype=f32).T),
        "cb": np.ascontiguousarray(cls_b, dtype=f32),
    }
    hs = np.asarray(hidden_states, dtype=f32)
    per_core = B // NCORES
    maps = []
    for c in range(NCORES):
        hT = np.ascontiguousarray(
            hs[c * per_core:(c + 1) * per_core].reshape(T, C).T)
        m = dict(shared)
        m["hT"] = hT
        maps.append(m)
    return maps


def kernel(**inputs) -> np.ndarray:
    nc = _get_nc()
    maps = _prep_maps(**inputs)
    res = bass_utils.run_bass_kernel_spmd(nc, maps, core_ids=list(range(NCORES)))
    outs = [res.results[c]["out"] for c in range(NCORES)]
    full = np.concatenate(outs, axis=0).reshape(B, S, L)
    return full.astype(np.float32)


# revision 2
# speedup vs baseline: 2.7952x; 2.7952x over previous
"""Trainium2 Bass kernel for nn_BertMoEClassifier.

Full-input contract: kernel(**inputs) takes the unsharded numpy inputs and
returns the full [32, 512, 2] logits.  Internally: data-parallel over the
batch dim across 8 NeuronCores (4 batches = 2048 tokens per core), dense
8-expert MoE with combine-weight masking on-device, no collectives.

Shapes (hardcoded): B=32 S=512 C=3072 D=768 H=1024 E=8 K=2 L=2.

Numerics: the projection matmul runs as a 3-term split-precision fp32r
product (hi/lo decomposition, ~fp32 accuracy at 1 cycle/row) and the router
in full fp32 — the discrete top-2 routing amplifies tiny numeric
differences into expert flips.  The expert MLPs run in bf16 with fp32 PSUM
accumulation.
"""

from contextlib import ExitStack

import ml_dtypes
import numpy as np

import concourse.bacc as bacc
import concourse.bass as bass
import concourse.mybir as mybir
import concourse.tile as tile
from concourse import bass_utils
from concourse.masks import make_identity

F32 = mybir.dt.float32
F32R = mybir.dt.float32r
BF16 = mybir.dt.bfloat16
AF = mybir.ActivationFunctionType
OP = mybir.AluOpType

B, S, C, D, H, E, L = 32, 512, 3072, 768, 1024, 8, 2
NCORES = 8
T = (B // NCORES) * S            # 2048 tokens per core
NT = T // 128                    # 16 token tiles
KC = C // 128                    # 24 contraction chunks (proj)
KD = D // 128                    # 6 chunks of D
KH = H // 128                    # 8 chunks of H
NKG = 3                          # proj k-groups
KGS = KC // NKG                  # 8 k-chunks per group
NEG_BIG = -1.0e30
EPS = 1e-5

_CACHE = {}


def _bcast_row(h_ap, off, n):
    """AP broadcasting a DRAM row of n elements across 128 partitions."""
    return bass.AP(tensor=h_ap.tensor, offset=h_ap.offset + off, ap=[[0, 128], [1, n]])


def _build():
    nc = bacc.Bacc("TRN2", target_bir_lowering=False, debug=False)

    hTh_d = nc.dram_tensor("hTh", [C, T], F32, kind="ExternalInput")
    hTl_d = nc.dram_tensor("hTl", [C, T], F32, kind="ExternalInput")
    pwh_d = nc.dram_tensor("pwh", [C, D], F32, kind="ExternalInput")
    pwl_d = nc.dram_tensor("pwl", [C, D], F32, kind="ExternalInput")
    pb_d = nc.dram_tensor("pb", [D], F32, kind="ExternalInput")
    g1_d = nc.dram_tensor("g1", [D], F32, kind="ExternalInput")
    be1_d = nc.dram_tensor("be1", [D], F32, kind="ExternalInput")
    g2_d = nc.dram_tensor("g2", [D], F32, kind="ExternalInput")
    be2_d = nc.dram_tensor("be2", [D], F32, kind="ExternalInput")
    gw_d = nc.dram_tensor("gw", [128, KD, E], F32, kind="ExternalInput")
    gb_d = nc.dram_tensor("gb", [E], F32, kind="ExternalInput")
    w1_d = nc.dram_tensor("w1", [E, KD, 128, H], BF16, kind="ExternalInput")
    b1_d = nc.dram_tensor("b1", [128, E, KH], F32, kind="ExternalInput")
    w2_d = nc.dram_tensor("w2", [E, KH, 128, D], BF16, kind="ExternalInput")
    b2_d = nc.dram_tensor("b2", [E, D], F32, kind="ExternalInput")
    cwT_d = nc.dram_tensor("cwT", [L, D], F32, kind="ExternalInput")
    cb_d = nc.dram_tensor("cb", [L], F32, kind="ExternalInput")
    out_d = nc.dram_tensor("out", [T, L], F32, kind="ExternalOutput")

    with ExitStack() as ctx:
        tc = ctx.enter_context(tile.TileContext(nc))
        persist = ctx.enter_context(tc.tile_pool(name="persist", bufs=1))

        # ---- persistent tiles -------------------------------------------
        acc = [persist.tile([128, D], F32, name=f"acc{t}", tag=f"acc{t}")
               for t in range(NT)]
        seqT = [persist.tile([128, T], BF16, name=f"seqT{j}", tag=f"seqT{j}")
                for j in range(KD)]
        comb = [persist.tile([128, E], F32, name=f"comb{t}", tag=f"comb{t}")
                for t in range(NT)]
        pbb = persist.tile([128, D], F32, name="pbb", tag="pbb")
        g1b = persist.tile([128, D], F32, name="g1b", tag="g1b")
        be1b = persist.tile([128, D], F32, name="be1b", tag="be1b")
        ident = persist.tile([128, 128], F32, name="ident", tag="ident")
        gwsb = persist.tile([128, KD, E], F32, name="gwsb", tag="gwsb")
        gbb = persist.tile([128, E], F32, name="gbb", tag="gbb")
        b1sb = persist.tile([128, E, KH], F32, name="b1sb", tag="b1sb")
        b2sb = persist.tile([E, D], F32, name="b2sb", tag="b2sb")
        epst = persist.tile([128, 1], F32, name="epst", tag="epst")

        nc.sync.dma_start(out=pbb, in_=_bcast_row(pb_d.ap(), 0, D))
        nc.sync.dma_start(out=g1b, in_=_bcast_row(g1_d.ap(), 0, D))
        nc.sync.dma_start(out=be1b, in_=_bcast_row(be1_d.ap(), 0, D))
        nc.sync.dma_start(out=gwsb, in_=gw_d.ap())
        nc.sync.dma_start(out=gbb, in_=_bcast_row(gb_d.ap(), 0, E))
        nc.sync.dma_start(out=b1sb, in_=b1_d.ap())
        nc.sync.dma_start(out=b2sb, in_=b2_d.ap())
        nc.vector.memset(epst, EPS)
        make_identity(nc, ident)

        groups = [(0, 3), (3, 3), (6, 3), (9, 3), (12, 3), (15, 1)]

        # ====== Phase 1a: split-fp32r proj accumulation + LN1 + GELU =====
        with tc.tile_pool(name="p1pw", bufs=10) as pwpool, \
             tc.tile_pool(name="p1ht", bufs=6) as htpool, \
             tc.tile_pool(name="p1sm", bufs=4) as smpool, \
             tc.tile_pool(name="p1psA", bufs=3, space="PSUM") as psA, \
             tc.tile_pool(name="p1psB", bufs=3, space="PSUM") as psB:

            for kg in range(NKG):
                pwh = []
                pwl = []
                for ki in range(KGS):
                    k = kg * KGS + ki
                    th = pwpool.tile([128, D], F32R, name=f"pwh{k}", tag="pwh")
                    nc.sync.dma_start(
                        out=th,
                        in_=pwh_d.ap()[k * 128:(k + 1) * 128, :].bitcast(F32R))
                    tl = pwpool.tile([128, D], F32R, name=f"pwl{k}", tag="pwl")
                    nc.sync.dma_start(
                        out=tl,
                        in_=pwl_d.ap()[k * 128:(k + 1) * 128, :].bitcast(F32R))
                    pwh.append(th)
                    pwl.append(tl)

                for g0, gn in groups:
                    pa = {}
                    pb_ = {}
                    for t in range(g0, g0 + gn):
                        pa[t] = psA.tile([128, 512], F32, name=f"pa{kg}_{t}",
                                         tag="psA")
                        pb_[t] = psB.tile([128, 256], F32, name=f"pb{kg}_{t}",
                                          tag="psB")
                    for ki in range(KGS):
                        k = kg * KGS + ki
                        hh = htpool.tile([128, gn * 128], F32R,
                                         name=f"hh{kg}_{g0}_{ki}", tag="hth")
                        nc.sync.dma_start(
                            out=hh,
                            in_=hTh_d.ap()[k * 128:(k + 1) * 128,
                                           g0 * 128:(g0 + gn) * 128].bitcast(F32R))
                        hl = htpool.tile([128, gn * 128], F32R,
                                         name=f"hl{kg}_{g0}_{ki}", tag="htl")
                        nc.sync.dma_start(
                            out=hl,
                            in_=hTl_d.ap()[k * 128:(k + 1) * 128,
                                           g0 * 128:(g0 + gn) * 128].bitcast(F32R))
                        st = (ki == 0)
                        sp = (ki == KGS - 1)
                        for i, t in enumerate(range(g0, g0 + gn)):
                            lh = hh[:, i * 128:(i + 1) * 128]
                            ll = hl[:, i * 128:(i + 1) * 128]
                            nc.tensor.matmul(pa[t], lh, pwh[ki][:, 0:512],
                                             start=st, stop=False)
                            nc.tensor.matmul(pa[t], lh, pwl[ki][:, 0:512],
                                             start=False, stop=False)
                            nc.tensor.matmul(pa[t], ll, pwh[ki][:, 0:512],
                                             start=False, stop=sp)
                            nc.tensor.matmul(pb_[t], lh, pwh[ki][:, 512:768],
                                             start=st, stop=False)
                            nc.tensor.matmul(pb_[t], lh, pwl[ki][:, 512:768],
                                             start=False, stop=False)
                            nc.tensor.matmul(pb_[t], ll, pwh[ki][:, 512:768],
                                             start=False, stop=sp)

                    for t in range(g0, g0 + gn):
                        x = acc[t]
                        if kg == 0:
                            nc.vector.tensor_tensor(out=x[:, 0:512], in0=pa[t],
                                                    in1=pbb[:, 0:512], op=OP.add)
                            nc.vector.tensor_tensor(out=x[:, 512:768],
                                                    in0=pb_[t],
                                                    in1=pbb[:, 512:768],
                                                    op=OP.add)
                        else:
                            nc.vector.tensor_tensor(out=x[:, 0:512], in0=pa[t],
                                                    in1=x[:, 0:512], op=OP.add)
                            nc.vector.tensor_tensor(out=x[:, 512:768],
                                                    in0=pb_[t],
                                                    in1=x[:, 512:768], op=OP.add)
                        if kg == NKG - 1:
                            # LN1 + GELU (DVE/ACT only; PE streams on)
                            stats = smpool.tile([128, 3, 6], F32,
                                                name=f"st{t}", tag="stats")
                            for sg in range(3):
                                nc.vector.bn_stats(
                                    out=stats[:, sg, :],
                                    in_=x[:, sg * 256:(sg + 1) * 256])
                            mv = smpool.tile([128, 2], F32, name=f"mv{t}",
                                             tag="mv")
                            nc.vector.bn_aggr(out=mv, in_=stats)
                            sd = smpool.tile([128, 1], F32, name=f"sd{t}",
                                             tag="sd")
                            nc.scalar.activation(out=sd, in_=mv[:, 1:2],
                                                 func=AF.Sqrt, bias=epst,
                                                 scale=1.0)
                            rstd = smpool.tile([128, 1], F32, name=f"rs{t}",
                                               tag="rstd")
                            nc.vector.reciprocal(out=rstd, in_=sd)
                            nc.vector.tensor_scalar(out=x, in0=x,
                                                    scalar1=mv[:, 0:1],
                                                    scalar2=rstd,
                                                    op0=OP.subtract,
                                                    op1=OP.mult)
                            nc.vector.tensor_tensor(out=x, in0=x, in1=g1b,
                                                    op=OP.mult)
                            nc.vector.tensor_tensor(out=x, in0=x, in1=be1b,
                                                    op=OP.add)
                            nc.scalar.activation(out=x, in_=x, func=AF.Gelu)

        # ====== Phase 1b: transpose to seqT + router + top-2 combine =====
        with tc.tile_pool(name="p1bst", bufs=2) as stgpool, \
             tc.tile_pool(name="p1bsm", bufs=4) as smpool, \
             tc.tile_pool(name="p1bv", bufs=6) as vpool, \
             tc.tile_pool(name="p1bps", bufs=3, space="PSUM") as psT:

            for t in range(NT):
                x = acc[t]
                stg = stgpool.tile([128, KD, 128], F32, name=f"stg{t}",
                                   tag="stg")
                for j in range(KD):
                    pt = psT.tile([128, 128], F32, name=f"pt{t}_{j}", tag="psT")
                    nc.tensor.transpose(pt, x[:, j * 128:(j + 1) * 128], ident)
                    nc.scalar.copy(out=stg[:, j, :], in_=pt)
                    nc.vector.tensor_copy(
                        out=seqT[j][:, t * 128:(t + 1) * 128],
                        in_=stg[:, j, :])

                pr = psT.tile([128, E], F32, name=f"pr{t}", tag="psT")
                for j in range(KD):
                    nc.tensor.matmul(pr, stg[:, j, :], gwsb[:, j, :],
                                     start=(j == 0), stop=(j == KD - 1))
                lg = vpool.tile([128, E], F32, name=f"lg{t}", tag="lg")
                nc.vector.tensor_tensor(out=lg, in0=pr, in1=gbb, op=OP.add)
                m1 = smpool.tile([128, 1], F32, name=f"m1{t}", tag="m1")
                nc.vector.reduce_max(out=m1, in_=lg, axis=mybir.AxisListType.X)
                nm1 = smpool.tile([128, 1], F32, name=f"nm1{t}", tag="nm1")
                nc.vector.tensor_scalar_mul(out=nm1, in0=m1, scalar1=-1.0)
                ea = vpool.tile([128, E], F32, name=f"ea{t}", tag="ea")
                nc.scalar.activation(out=ea, in_=lg, func=AF.Exp, bias=nm1,
                                     scale=1.0)
                mm = vpool.tile([128, E], F32, name=f"mm{t}", tag="mm")
                nc.vector.tensor_scalar(out=mm, in0=lg, scalar1=m1,
                                        scalar2=None, op0=OP.is_ge)
                lg2 = vpool.tile([128, E], F32, name=f"lg2{t}", tag="lg2")
                nc.vector.scalar_tensor_tensor(out=lg2, in0=mm, scalar=NEG_BIG,
                                               in1=lg, op0=OP.mult, op1=OP.add)
                m2 = smpool.tile([128, 1], F32, name=f"m2{t}", tag="m2")
                nc.vector.reduce_max(out=m2, in_=lg2, axis=mybir.AxisListType.X)
                mk2 = vpool.tile([128, E], F32, name=f"mk2{t}", tag="mk2")
                nc.vector.tensor_scalar(out=mk2, in0=lg, scalar1=m2,
                                        scalar2=None, op0=OP.is_ge)
                p2 = vpool.tile([128, E], F32, name=f"p2{t}", tag="p2")
                nc.vector.tensor_mul(out=p2, in0=ea, in1=mk2)
                sm = smpool.tile([128, 1], F32, name=f"sm{t}", tag="sm")
                nc.vector.reduce_sum(out=sm, in_=p2, axis=mybir.AxisListType.X)
                rsm = smpool.tile([128, 1], F32, name=f"rsm{t}", tag="rsm")
                nc.vector.reciprocal(out=rsm, in_=sm)
                nc.vector.tensor_scalar_mul(out=comb[t], in0=p2, scalar1=rsm)

        # ================= Phase 2: dense 8-expert MoE ===================
        with tc.tile_pool(name="p2w1", bufs=8) as w1pool, \
             tc.tile_pool(name="p2w2", bufs=10) as w2pool, \
             tc.tile_pool(name="p2h", bufs=18) as hpool, \
             tc.tile_pool(name="p2psA", bufs=3, space="PSUM") as psA2, \
             tc.tile_pool(name="p2psE", bufs=2, space="PSUM") as psE, \
             tc.tile_pool(name="p2psB", bufs=2, space="PSUM") as psB2:

            for e in range(E):
                w1t = []
                for k in range(KD):
                    w = w1pool.tile([128, H], BF16, name=f"w1_{e}_{k}", tag="w1")
                    nc.sync.dma_start(out=w, in_=w1_d.ap()[e, k])
                    w1t.append(w)
                w2t = []
                for k in range(KH):
                    w = w2pool.tile([128, D], BF16, name=f"w2_{e}_{k}", tag="w2")
                    nc.sync.dma_start(out=w, in_=w2_d.ap()[e, k])
                    w2t.append(w)

                for n in range(T // 512):
                    htiles = []
                    for m in range(KH):
                        ps = psA2.tile([128, 512], F32, name=f"ph{e}_{n}_{m}",
                                       tag="psA2")
                        for k in range(KD):
                            nc.tensor.matmul(
                                ps, w1t[k][:, m * 128:(m + 1) * 128],
                                seqT[k][:, n * 512:(n + 1) * 512],
                                start=(k == 0), stop=(k == KD - 1))
                        h = hpool.tile([128, 512], BF16, name=f"h{e}_{n}_{m}",
                                       tag="h")
                        nc.scalar.activation(out=h, in_=ps, func=AF.Gelu,
                                             bias=b1sb[:, e:e + 1, m:m + 1],
                                             scale=1.0)
                        htiles.append(h)
                    for ti in range(4):
                        t = n * 4 + ti
                        pea = psE.tile([128, 512], F32, name=f"pea{e}_{t}",
                                       tag="psE")
                        peb = psB2.tile([128, 256], F32, name=f"peb{e}_{t}",
                                        tag="psB2")
                        for k in range(KH):
                            lhs = htiles[k][:, ti * 128:(ti + 1) * 128]
                            nc.tensor.matmul(pea, lhs, w2t[k][:, 0:512],
                                             start=(k == 0), stop=(k == KH - 1))
                            nc.tensor.matmul(peb, lhs, w2t[k][:, 512:768],
                                             start=(k == 0), stop=(k == KH - 1))
                        c = comb[t][:, e:e + 1]
                        nc.vector.scalar_tensor_tensor(
                            out=acc[t][:, 0:512], in0=pea, scalar=c,
                            in1=acc[t][:, 0:512], op0=OP.mult, op1=OP.add)
                        nc.vector.scalar_tensor_tensor(
                            out=acc[t][:, 512:768], in0=peb, scalar=c,
                            in1=acc[t][:, 512:768], op0=OP.mult, op1=OP.add)

        # ================= Phase 3: +combine@b2, LN2, classifier =========
        with tc.tile_pool(name="p3", bufs=2) as p3pool, \
             tc.tile_pool(name="p3sm", bufs=4) as sm3, \
             tc.tile_pool(name="p3out", bufs=4) as outpool, \
             tc.tile_pool(name="p3psT", bufs=2, space="PSUM") as psT3, \
             tc.tile_pool(name="p3psA", bufs=2, space="PSUM") as psA3, \
             tc.tile_pool(name="p3psB", bufs=2, space="PSUM") as psB3:

            g2b = p3pool.tile([128, D], F32, name="g2b", tag="g2b", bufs=1)
            be2b = p3pool.tile([128, D], F32, name="be2b", tag="be2b", bufs=1)
            cw0 = p3pool.tile([128, D], F32, name="cw0", tag="cw0", bufs=1)
            cw1 = p3pool.tile([128, D], F32, name="cw1", tag="cw1", bufs=1)
            cbb = p3pool.tile([128, L], F32, name="cbb", tag="cbb", bufs=1)
            nc.sync.dma_start(out=g2b, in_=_bcast_row(g2_d.ap(), 0, D))
            nc.sync.dma_start(out=be2b, in_=_bcast_row(be2_d.ap(), 0, D))
            nc.sync.dma_start(out=cw0, in_=_bcast_row(cwT_d.ap(), 0, D))
            nc.sync.dma_start(out=cw1, in_=_bcast_row(cwT_d.ap(), D, D))
            nc.sync.dma_start(out=cbb, in_=_bcast_row(cb_d.ap(), 0, L))

            for t in range(NT):
                x = acc[t]
                # acc += combine[t] @ b2   (fold per-expert output bias)
                ptc = psT3.tile([E, 128], F32, name=f"ptc{t}", tag="psT3")
                nc.tensor.transpose(ptc, comb[t], ident)
                cT = sm3.tile([E, 128], F32, name=f"cT{t}", tag="cT")
                nc.scalar.copy(out=cT, in_=ptc)
                pca = psA3.tile([128, 512], F32, name=f"pca{t}", tag="psA3")
                pcb = psB3.tile([128, 256], F32, name=f"pcb{t}", tag="psB3")
                nc.tensor.matmul(pca, cT, b2sb[:, 0:512], start=True, stop=True)
                nc.tensor.matmul(pcb, cT, b2sb[:, 512:768], start=True, stop=True)
                nc.vector.scalar_tensor_tensor(out=x[:, 0:512], in0=pca,
                                               scalar=1.0, in1=x[:, 0:512],
                                               op0=OP.mult, op1=OP.add)
                nc.vector.scalar_tensor_tensor(out=x[:, 512:768], in0=pcb,
                                               scalar=1.0, in1=x[:, 512:768],
                                               op0=OP.mult, op1=OP.add)
                # LN2
                stats = sm3.tile([128, 3, 6], F32, name=f"s3{t}", tag="s3")
                for sg in range(3):
                    nc.vector.bn_stats(out=stats[:, sg, :],
                                       in_=x[:, sg * 256:(sg + 1) * 256])
                mv = sm3.tile([128, 2], F32, name=f"mv3{t}", tag="mv3")
                nc.vector.bn_aggr(out=mv, in_=stats)
                sd = sm3.tile([128, 1], F32, name=f"sd3{t}", tag="sd3")
                nc.scalar.activation(out=sd, in_=mv[:, 1:2], func=AF.Sqrt,
                                     bias=epst, scale=1.0)
                rstd = sm3.tile([128, 1], F32, name=f"rs3{t}", tag="rs3")
                nc.vector.reciprocal(out=rstd, in_=sd)
                nc.vector.tensor_scalar(out=x, in0=x, scalar1=mv[:, 0:1],
                                        scalar2=rstd, op0=OP.subtract,
                                        op1=OP.mult)
                nc.vector.tensor_tensor(out=x, in0=x, in1=g2b, op=OP.mult)
                nc.vector.tensor_tensor(out=x, in0=x, in1=be2b, op=OP.add)
                # logits = y @ cls_w + cls_b  (DVE dot products)
                lt = outpool.tile([128, L], F32, name=f"lt{t}", tag="lt")
                scr = p3pool.tile([128, D], F32, name=f"scr{t}", tag="scr")
                nc.vector.scalar_tensor_tensor(out=scr, in0=x, scalar=1.0,
                                               in1=cw0, op0=OP.mult,
                                               op1=OP.mult,
                                               accum_out=lt[:, 0:1])
                scr2 = p3pool.tile([128, D], F32, name=f"scr2{t}", tag="scr2")
                nc.vector.scalar_tensor_tensor(out=scr2, in0=x, scalar=1.0,
                                               in1=cw1, op0=OP.mult,
                                               op1=OP.mult,
                                               accum_out=lt[:, 1:2])
                nc.vector.tensor_tensor(out=lt, in0=lt, in1=cbb, op=OP.add)
                nc.sync.dma_start(out=out_d.ap()[t * 128:(t + 1) * 128, :],
                                  in_=lt)

    nc.compile()
    nc.finalize()
    return nc


def _get_nc():
    if "nc" not in _CACHE:
        _CACHE["nc"] = _build()
    return _CACHE["nc"]


def _round_bits(a, nbits):
    """Round fp32 array to nbits explicit mantissa bits (round-to-nearest)."""
    u = a.view(np.uint32)
    shift = 23 - nbits
    half = np.uint32(1 << (shift - 1))
    mask = np.uint32(~((1 << shift) - 1) & 0xFFFFFFFF)
    return ((u + half) & mask).view(np.float32)


def _prep_maps(hidden_states, proj_w, proj_b, ln1_g, ln1_b, gate_w, gate_b,
               w1, b1, w2, b2, ln2_g, ln2_b, cls_w, cls_b):
    bf16 = ml_dtypes.bfloat16
    f32 = np.float32
    pw = np.ascontiguousarray(proj_w, dtype=f32)
    pwh = _round_bits(pw, 10)
    pwl = pw - pwh
    shared = {
        "pwh": pwh,
        "pwl": pwl,
        "pb": np.ascontiguousarray(proj_b, dtype=f32),
        "g1": np.ascontiguousarray(ln1_g, dtype=f32),
        "be1": np.ascontiguousarray(ln1_b, dtype=f32),
        "g2": np.ascontiguousarray(ln2_g, dtype=f32),
        "be2": np.ascontiguousarray(ln2_b, dtype=f32),
        # gate_w [D,E] -> [128, KD, E]
        "gw": np.ascontiguousarray(
            np.asarray(gate_w, dtype=f32).reshape(KD, 128, E).transpose(1, 0, 2)),
        "gb": np.ascontiguousarray(gate_b, dtype=f32),
        # w1 [E,D,H] -> [E, KD, 128, H] bf16
        "w1": np.ascontiguousarray(
            np.asarray(w1).reshape(E, KD, 128, H)).astype(bf16),
        # b1 [E,H] -> [128, E, KH]
        "b1": np.ascontiguousarray(
            np.asarray(b1, dtype=f32).reshape(E, KH, 128).transpose(2, 0, 1)),
        # w2 [E,H,D] -> [E, KH, 128, D] bf16
        "w2": np.ascontiguousarray(
            np.asarray(w2).reshape(E, KH, 128, D)).astype(bf16),
        "b2": np.ascontiguousarray(b2, dtype=f32),
        "cwT": np.ascontiguousarray(np.asarray(cls_w, dtype=f32).T),
        "cb": np.ascontiguousarray(cls_b, dtype=f32),
    }
    hs = np.asarray(hidden_states, dtype=f32)
    per_core = B // NCORES
    maps = []
    for c in range(NCORES):
        hT = np.ascontiguousarray(
            hs[c * per_core:(c + 1) * per_core].reshape(T, C).T)
        hTh = _round_bits(hT, 10)
        hTl = hT - hTh
        m = dict(shared)
        m["hTh"] = hTh
        m["hTl"] = hTl
        maps.append(m)
    return maps


def kernel(**inputs) -> np.ndarray:
    nc = _get_nc()
    maps = _prep_maps(**inputs)
    res = bass_utils.run_bass_kernel_spmd(nc, maps, core_ids=list(range(NCORES)))
    outs = [res.results[c]["out"] for c in range(NCORES)]
    full = np.concatenate(outs, axis=0).reshape(B, S, L)
    return full.astype(np.float32)
